# revision 63
# baseline (speedup 1.0000x reference)
"""Trainium2 Bass kernel: dynamic deformable propagation (6 iterations).

v2 rewrite of the staged baseline. Same math (25-cell merged stencil per
deform conv, stale-halo column split, row-block fold), restructured for
DMA/sync efficiency:
  - all dtype conversion on host; zero gpsimd software-DGE DMAs
  - channel-interleaved host layouts -> few, large hardware-DGE DMAs
  - conv as single 120-contraction matmul per row-triple (dx folded into
    the stationary), 3x less PE time than psum-accumulated triples
  - no HBM staging of per-iteration fields: E gates + P/Q/R computed
    inline each iteration from f16 dyn loads
  - C1 + C2 center row resident in SBUF; C2's four off-center dy-groups
    round-trip HBM once and stream back double-buffered per iteration

Sharding: one core per (image, x-half): 480 rows, 320 own cols + 12-col
stale halo. Rows on partitions, 4 row-blocks of 124 folded along free dim.
"""
import sys, types

sys.path.insert(0, '/opt/trn_rl_repo')
import numpy as np


def _install_hook():
    try:
        import antenv
        if not hasattr(antenv, 'axon_hooks'):
            mod = types.ModuleType("antenv.axon_hooks")
            _h = [None]
            mod.set_axon_ntff_profile_hook = lambda h: _h.__setitem__(0, h)
            mod.get_axon_ntff_profile_hook = lambda: _h[0]
            sys.modules["antenv.axon_hooks"] = mod
            antenv.axon_hooks = mod
            from trn_agent_boot.trn_boot import _ntff_profile_via_ctypes
            mod.set_axon_ntff_profile_hook(
                _ntff_profile_via_ctypes('/opt/axon/libaxon_pjrt.so'))
    except Exception:
        pass


_install_hook()

import concourse.bass as bass
import concourse.mybir as mybir
from concourse.tile import TileContext
from concourse import bass_utils

AF = mybir.ActivationFunctionType
OP = mybir.AluOpType
dt = mybir.dt

B, H, W = 4, 480, 640
PROP = 6
NCORE = 8
ROWS, BW, NB = 512, 336, 4
XF = NB * BW              # 1344
XA, XB = 1, 335           # conv / C-field col region
XWID = XB - XA            # 334
X2A, X2B = 2, 334         # owned + stale-halo write region
F16, F32, BF16 = dt.float16, dt.float32, dt.bfloat16
XL = XF - 4               # 1340: full-width op region, reads cover [0, XF)
HXF = XF // 2             # 672: half-width C2 stream granule
TRI = 42                  # row-triples per block (3*42 = 126 rows)
TB = 14                   # triples per slab batch (3 batches per block)
NBT = 3

SH = [(1, 1), (1, 0), (1, -1), (0, 1), (0, -1), (-1, 1), (-1, 0), (-1, -1)]
TAPS = [j for j in range(9) if j != 4]
G4 = [0, 1, 3, 4]         # streamed C2 dy-groups (2 = center, resident)

# engine split knobs: 1-in-N tensor_tensor ops go to gpsimd
IT_GPS_MOD = 3
CB_GPS_MOD = 4


def _reord(v, *order):
    cur = [list(p) for p in v.ap]
    for i, o in enumerate(order):
        v.ap[i] = cur[o]
    return v


def _fwin(t, pa, pb, n, width, base=0):
    """Overlapping window view [pb-pa, n, width]; element (c, x) at col base+c+x."""
    v = t[pa:pb, base:base + width].unsqueeze(1)
    v.ap[1] = [1, n]
    return v


def _pack_conv120(w, bi):
    """Stationary [120=(d,c,j), 81=(s,q,t)] for single-matmul conv triples."""
    Wm = np.zeros((120, 81), np.float32)
    b81 = np.zeros((81, 1), np.float32)
    for s in range(3):
        for t2 in range(9):
            if t2 == 4:
                continue
            idx = TAPS.index(t2)
            for q in range(3):
                oref = 2 * idx if q == 0 else (2 * idx + 1 if q == 1 else 16 + idx)
                o81 = s * 27 + q * 9 + t2
                b81[o81, 0] = bi[oref]
                for d in range(3):
                    for c in range(8):
                        for j in range(5):
                            ky = j - s
                            if 0 <= ky <= 2:
                                Wm[d * 40 + c * 5 + j, o81] = w[oref, c, ky, d]
    return Wm, b81


def _split_2d_f16(nc):
    # BIR verifier rejects 2-free-dim 2-byte compute APs at partition start>0;
    # equivalent 3D APs pass. Split last dim [1, n] -> [n//2, 2], [1, n//2].
    nsp = 0
    for f in nc.m.functions:
        for blk in f.blocks:
            for inst in blk.instructions:
                if type(inst).__name__ not in ("InstTensorTensor",
                                               "InstTensorCopy"):
                    continue
                for arg in list(inst.ins) + list(inst.outs):
                    ap = getattr(arg, 'ap', None)
                    dtp = getattr(arg, 'dtype', None)
                    if ap is None or dtp is None:
                        continue
                    try:
                        dsz = mybir.dt.np(dtp)().itemsize
                    except Exception:
                        continue
                    if (dsz == 2 and len(ap) == 2 and ap[1][0] == 1
                            and ap[1][1] % 2 == 0 and ap[1][1] >= 2):
                        n = ap[1][1]
                        arg.ap = [list(ap[0]), [n // 2, 2], [1, n // 2]]
                        nsp += 1
    return nsp


def _split_waits(nc, maxw=1):
    n_split = 0
    for f in nc.m.functions:
        for blk in f.blocks:
            out_list = []
            changed = False
            for inst in blk.instructions:
                si = inst.sync_info
                if si is not None and len(si.on_wait) > maxw:
                    waits = list(si.on_wait)
                    extra, keep = waits[:-maxw], waits[-maxw:]
                    for w_i, w in enumerate(extra):
                        nop = mybir.InstNoOp(name=f"{inst.name}-w{w_i}",
                                             ins=[], outs=[])
                        nop.engine = inst.engine
                        nop.sync_info = mybir.SyncInfo(on_wait=[w], on_update=[])
                        out_list.append(nop)
                        n_split += 1
                    si.on_wait = keep
                    inst.sync_info = si
                    changed = True
                out_list.append(inst)
            if changed:
                blk.instructions = out_list
    return n_split


def build_nc():
    nc = bass.Bass(trn_type="TRN2")
    for val in (1e-4,):
        _t = nc.alloc_sbuf_tensor(f"const-f32-{val}", [128, 1], F32)
        nc.gpsimd.memset(_t.ap(), val)
        nc.const_aps.aps[(F32, val)] = _t.ap()
    nc.all_engine_barrier()

    g12D = nc.dram_tensor("g12", [3, 80, TRI * NB, XWID], BF16,
                          kind="ExternalInput")
    g3D = nc.dram_tensor("g3s", [ROWS, 8, BW], F16, kind="ExternalInput")
    dyD = nc.dram_tensor("dyn", [PROP, ROWS, 4, BW], F16, kind="ExternalInput")
    fcD = nc.dram_tensor("fcf", [ROWS, 3, BW], F32, kind="ExternalInput")
    fhD = nc.dram_tensor("fnh", [ROWS, BW], F16, kind="ExternalInput")
    w1D = nc.dram_tensor("w1", [120, 81], BF16, kind="ExternalInput")
    w2D = nc.dram_tensor("w2", [120, 81], BF16, kind="ExternalInput")
    b1D = nc.dram_tensor("b1", [81, 1], F32, kind="ExternalInput")
    b2D = nc.dram_tensor("b2", [81, 1], F32, kind="ExternalInput")
    outD = nc.dram_tensor("out", [480, 332], F16, kind="ExternalOutput")
    c2sD = nc.dram_tensor("c2s", [4, 2, 124, 5, HXF], F16)

    cnt = [0]

    def addeng(mod):
        cnt[0] += 1
        return nc.gpsimd if (cnt[0] % mod == 0) else nc.vector

    dcnt = [0]

    def dmaeng():
        dcnt[0] += 1
        return nc.scalar if (dcnt[0] % 2 == 0) else nc.sync

    with nc.allow_low_precision("deform propagation is f16 end-to-end"), \
         TileContext(nc) as tc:
        with tc.tile_pool(name="po", bufs=1) as po:
            C1 = po.tile([128, 25, XF], F16, tag="C1")
            C2c = po.tile([128, 5, XF], F16, tag="C2c")
            Afl = po.tile([128, 6, XF], F16, tag="Afl")
            alpT = po.tile([128, XF], F16, tag="alpT")
            finT = po.tile([128, XF], F16, tag="finT")
            betT = po.tile([128, XF], F16, tag="betT")
            faT = po.tile([128, XF], F16, tag="faT")
            fbT = po.tile([128, XF], F16, tag="fbT")
            wT = [po.tile([120, 81], BF16, tag=f"wT{cv}", name=f"wT{cv}")
                  for cv in range(2)]
            bT = [po.tile([81, 1], F32, tag=f"bT{cv}", name=f"bT{cv}")
                  for cv in range(2)]

            for cv, (wD, bD) in enumerate(((w1D, b1D), (w2D, b2D))):
                nc.sync.dma_start(out=wT[cv][:, :], in_=wD[:, :])
                nc.sync.dma_start(out=bT[cv][:, :], in_=bD[:, :])
            for b in range(NB):
                bs = 124 * b
                nc.scalar.dma_start(out=faT[:, b * BW:(b + 1) * BW],
                                    in_=fhD[bs:bs + 128, :])
                nc.scalar.dma_start(out=fbT[:, b * BW:(b + 1) * BW],
                                    in_=fhD[bs:bs + 128, :])
                nc.sync.dma_start(out=finT[0:124, b * BW:(b + 1) * BW],
                                  in_=fhD[bs + 2:bs + 126, :])
            nc.gpsimd.memset(C1[:, :, :], 0.0)
            nc.gpsimd.memset(C2c[:, :, :], 0.0)
            nc.gpsimd.memset(Afl[:, :, :], 0.0)

            # ---- early: alp/bet from conf/fix; A2/A5 from g3 ----
            with tc.tile_pool(name="pe", bufs=1) as pe:
                fcT = pe.tile([128, 3, XF], F32, tag="fcT")
                sgT = pe.tile([128, XF], F32, tag="sgT")
                snT = pe.tile([128, XF], F32, tag="snT")
                agT = pe.tile([128, 8, XF], F16, tag="agT")
                for b in range(NB):
                    bs = 124 * b
                    nc.sync.dma_start(out=fcT[0:124, :, b * BW:(b + 1) * BW],
                                      in_=fcD[bs + 2:bs + 126, :, :])
                    nc.sync.dma_start(out=agT[0:124, :, b * BW:(b + 1) * BW],
                                      in_=g3D[bs + 2:bs + 126, :, :])
                cnfv = fcT[0:124, 1, :]
                ffxv = fcT[0:124, 2, :]
                nc.scalar.activation(out=sgT[0:124, :], in_=cnfv, func=AF.Sigmoid)
                nc.scalar.activation(out=snT[0:124, :], in_=ffxv, func=AF.Sign)
                nc.vector.tensor_tensor(out=sgT[0:124, :], in0=sgT[0:124, :],
                                        in1=snT[0:124, :], op=OP.mult)
                nc.scalar.activation(out=alpT[0:124, :], in_=sgT[0:124, :],
                                     func=AF.Identity, scale=-1.0, bias=1.0)
                nc.vector.tensor_tensor(out=betT[0:124, :], in0=sgT[0:124, :],
                                        in1=ffxv, op=OP.mult)
                for k6 in (2, 5):
                    if k6 == 5:
                        nc.scalar.activation(out=agT[0:124, :, :],
                                             in_=agT[0:124, :, :], func=AF.Abs)
                    nc.vector.tensor_tensor(out=Afl[0:124, k6, :],
                                            in0=agT[0:124, 0, :],
                                            in1=agT[0:124, 1, :], op=OP.add)
                    for i in range(2, 8):
                        addeng(CB_GPS_MOD).tensor_tensor(
                            out=Afl[0:124, k6, :], in0=Afl[0:124, k6, :],
                            in1=agT[0:124, i, :], op=OP.add)

            # ---- conv + C build ----
            with tc.tile_pool(name="pb", bufs=1) as pb, \
                 tc.tile_pool(name="p2", bufs=2) as p2, \
                 tc.tile_pool(name="pps", bufs=8, space="PSUM") as pps:
                for ib in range(NB):
                    bs = 124 * ib
                    xb0 = ib * BW
                    for cv in range(2):
                        oa = pb.tile([128, 27, BW], F16, tag="oa", bufs=2)
                        for bt in range(NBT):
                            slab = pb.tile([120, TB, XWID], BF16, tag="slab")
                            t0 = TRI * ib + bt * TB
                            for d in range(3):
                                nc.sync.dma_start(
                                    out=slab[40 * d:40 * d + 40, :, :],
                                    in_=g12D[d, 40 * cv:40 * cv + 40,
                                             t0:t0 + TB, :])
                            for tq in range(0, TB, 4):
                                tn = min(4, TB - tq)
                                psb = pps.tile([81, 4, 512], F32, tag="ps",
                                               name="psb", bufs=2)
                                for t in range(tq, tq + tn):
                                    nc.tensor.matmul(psb[:, t - tq, 0:XWID],
                                                     wT[cv][:, :],
                                                     slab[:, t, :],
                                                     start=True, stop=True)
                                est = p2.tile([81, 4, XWID], F16, tag="est")
                                nc.scalar.activation(
                                    out=est[:, 0:tn, :],
                                    in_=psb[:, 0:tn, 0:XWID],
                                    func=AF.Identity,
                                    bias=bT[cv][:, :], scale=1.0)
                                for t in range(tq, tq + tn):
                                    pr0 = 3 * (bt * TB + t)
                                    dmaeng().dma_start(
                                        out=oa[pr0:pr0 + 3, :, XA:XB],
                                        in_=est[:, t - tq, :])
                        # ---- C build (half-block width ops) ----
                        mv = oa[0:124, 18:27, XA:XB]
                        HB = (XA + XB) // 2
                        w9 = {nm: pb.tile([128, 9, HB], F16, tag=f"w9{nm}",
                                          name=f"w9{nm}")
                              for nm in ("ay", "by", "cy", "ax", "bx", "cx",
                                         "ry", "p9")}
                        stg = (pb.tile([128, 20, BW], F16, tag="stg",
                                       name="stg")
                               if cv == 1 else None)
                        if cv == 1:
                            nc.gpsimd.memset(stg[:, :, :], 0.0)
                        # ---- A-field m sums ----
                        nc.vector.tensor_tensor(
                            out=Afl[0:124, cv, xb0 + XA:xb0 + XB],
                            in0=oa[0:124, 18, XA:XB],
                            in1=oa[0:124, 19, XA:XB], op=OP.add)
                        for t2 in range(2, 9):
                            addeng(CB_GPS_MOD).tensor_tensor(
                                out=Afl[0:124, cv, xb0 + XA:xb0 + XB],
                                in0=Afl[0:124, cv, xb0 + XA:xb0 + XB],
                                in1=oa[0:124, 18 + t2, XA:XB], op=OP.add)
                        aam = pb.tile([128, 9, BW], F16, tag="aam")
                        nc.scalar.activation(out=aam[0:124, :, XA:XB], in_=mv,
                                             func=AF.Abs)
                        nc.vector.tensor_tensor(
                            out=Afl[0:124, 3 + cv, xb0 + XA:xb0 + XB],
                            in0=aam[0:124, 0, XA:XB],
                            in1=aam[0:124, 1, XA:XB], op=OP.add)
                        for t2 in range(2, 9):
                            addeng(CB_GPS_MOD).tensor_tensor(
                                out=Afl[0:124, 3 + cv, xb0 + XA:xb0 + XB],
                                in0=Afl[0:124, 3 + cv, xb0 + XA:xb0 + XB],
                                in1=aam[0:124, t2, XA:XB], op=OP.add)
                        wyl = ("by", "cy", "ay")
                        wxl = ("bx", "cx", "ax")
                        for x0, x1 in ((XA, XA + HB), (XA + HB, XB)):
                            hw_ = x1 - x0
                            ty = oa[0:124, 0:9, x0:x1]
                            tx = oa[0:124, 9:18, x0:x1]
                            mh = oa[0:124, 18:27, x0:x1]
                            for (src, a_, b_, c_) in ((ty, "ay", "by", "cy"),
                                                      (tx, "ax", "bx", "cx")):
                                A_ = w9[a_][0:124, :, 0:hw_]
                                B_ = w9[b_][0:124, :, 0:hw_]
                                C_ = w9[c_][0:124, :, 0:hw_]
                                nc.scalar.activation(out=A_, in_=src,
                                                     func=AF.Relu)
                                nc.scalar.activation(out=B_, in_=src,
                                                     func=AF.Relu, scale=-1.0)
                                nc.vector.tensor_tensor(out=C_, in0=A_, in1=B_,
                                                        op=OP.add)
                                nc.scalar.activation(out=C_, in_=C_,
                                                     func=AF.Identity,
                                                     scale=-1.0, bias=1.0)
                            ryv = w9["ry"][0:124, :, 0:hw_]
                            p9v = w9["p9"][0:124, :, 0:hw_]
                            for i in range(3):
                                nc.vector.tensor_tensor(
                                    out=ryv, in0=mh,
                                    in1=w9[wyl[i]][0:124, :, 0:hw_], op=OP.mult)
                                for jj in range(3):
                                    nc.vector.tensor_tensor(
                                        out=p9v, in0=ryv,
                                        in1=w9[wxl[jj]][0:124, :, 0:hw_],
                                        op=OP.mult)
                                    for ky in range(3):
                                        c0 = (ky + i) * 5 + jj
                                        srcv = w9["p9"][0:124,
                                                        3 * ky:3 * ky + 3,
                                                        0:hw_]
                                        if cv == 0:
                                            dstv = C1[0:124, c0:c0 + 3,
                                                      xb0 + x0:xb0 + x1]
                                        else:
                                            g = c0 // 5
                                            cc = c0 % 5
                                            if g == 2:
                                                dstv = C2c[0:124, cc:cc + 3,
                                                           xb0 + x0:xb0 + x1]
                                            else:
                                                g4 = G4.index(g)
                                                dstv = stg[
                                                    0:124,
                                                    5 * g4 + cc:5 * g4 + cc + 3,
                                                    x0:x1]
                                        addeng(CB_GPS_MOD).tensor_tensor(
                                            out=dstv, in0=dstv, in1=srcv,
                                            op=OP.add)
                        if cv == 1:
                            xh0 = (ib % 2) * BW
                            for g4 in range(4):
                                dmaeng().dma_start(
                                    out=c2sD[g4, ib // 2, :, :,
                                             xh0:xh0 + BW],
                                    in_=stg[0:124, 5 * g4:5 * g4 + 5, :])
                nc.scalar.activation(out=Afl[0:124, 3:6, :],
                                     in_=Afl[0:124, 3:6, :],
                                     func=AF.Identity, bias=1e-4)

            # ---- iterations ----
            with tc.tile_pool(name="pi", bufs=1) as pi, \
                 tc.tile_pool(name="pc2", bufs=2) as pc2:
                g3T = pi.tile([128, 8, XF], F16, tag="g3T")
                for b in range(NB):
                    bs = 124 * b
                    nc.sync.dma_start(out=g3T[0:124, :, b * BW:(b + 1) * BW],
                                      in_=g3D[bs + 2:bs + 126, :, :])
                Fs = [pi.tile([128, XF], F16, tag=f"Fs{s}", name=f"Fs{s}")
                      for s in range(1, 5)]
                u1 = pi.tile([128, XF], F16, tag="u1")
                u2 = pi.tile([128, XF], F16, tag="u2")
                num = pi.tile([128, XF], F16, tag="num")
                cmb = pi.tile([128, XF], F16, tag="cmb")
                PT = pi.tile([128, XF], F16, tag="PT")
                QT = pi.tile([128, XF], F16, tag="QT")
                TrT = pi.tile([128, XF], F16, tag="TrT")
                eT = pi.tile([128, 4, XF], F16, tag="eT")
                prod = [pi.tile([128, 5, XF], F16, tag=f"prod{i}",
                                name=f"prod{i}") for i in range(2)]

                cur, nxt = faT, fbT

                def FS(g):
                    return cur if g == 0 else Fs[g - 1]

                # prefetch iteration-0 inputs during the phase-1 tail
                for s in range(1, 5):
                    dmaeng().dma_start(out=Fs[s - 1][0:128 - s, :],
                                       in_=faT[s:128, :])
                for b in range(NB):
                    bs = 124 * b
                    dmaeng().dma_start(
                        out=eT[0:124, :, b * BW:(b + 1) * BW],
                        in_=dyD[0, bs + 2:bs + 126, :, :])

                tc.strict_bb_all_engine_barrier()

                for k in range(PROP):
                    if k > 0:
                        for s in range(1, 5):
                            dmaeng().dma_start(out=Fs[s - 1][0:128 - s, :],
                                               in_=cur[s:128, :])
                        for b in range(NB):
                            bs = 124 * b
                            dmaeng().dma_start(
                                out=eT[0:124, :, b * BW:(b + 1) * BW],
                                in_=dyD[k, bs + 2:bs + 126, :, :])
                    E = [eT[0:124, g, 2:2 + XL] for g in range(4)]
                    cs_t = {}

                    def cs_load(g, h):
                        lo, nct = (1, 3) if g in (0, 4) else (0, 5)
                        cs = pc2.tile([128, 5, HXF], F16, tag="cs", name="cs")
                        dmaeng().dma_start(
                            out=cs[0:124, 0:nct, :],
                            in_=c2sD[G4.index(g), h, :, lo:lo + nct, :])
                        cs_t[(g, h)] = cs

                    cs_load(0, 0)
                    cs_load(0, 1)

                    # u1 (C1 resident): 5 full-width groups
                    pcnt = [0]

                    def nprod():
                        pcnt[0] += 1
                        return prod[pcnt[0] % 2]

                    uv = u1[0:124, 2:2 + XL]
                    first = True
                    for g in range(5):
                        lo, nct = (1, 3) if g in (0, 4) else (0, 5)
                        Cw = C1[0:124, 5 * g + lo:5 * g + lo + nct, 2:2 + XL]
                        pr = nprod()
                        fw = _fwin(FS(g), 0, 124, nct, XL, base=lo)
                        nc.vector.tensor_tensor(
                            out=pr[0:124, 0:nct, 2:2 + XL], in0=Cw, in1=fw,
                            op=OP.mult)
                        ci5 = 0
                        if first:
                            nc.vector.tensor_tensor(
                                out=uv, in0=pr[0:124, 0, 2:2 + XL],
                                in1=pr[0:124, 1, 2:2 + XL], op=OP.add)
                            first = False
                            ci5 = 2
                        for ci in range(ci5, nct):
                            addeng(IT_GPS_MOD).tensor_tensor(
                                out=uv, in0=uv,
                                in1=pr[0:124, ci, 2:2 + XL], op=OP.add)

                    # u3 into TrT: batched per sdy-group (host channel order)
                    u3v = TrT[0:124, 2:2 + XL]
                    Cv = cmb[0:124, 2:2 + XL]
                    first = True
                    for (c0g, ncg, fs_i, stride) in ((0, 3, 3, 1), (3, 2, 2, 2),
                                                     (5, 3, 1, 1)):
                        pr = nprod()
                        fw = _fwin(FS(fs_i), 0, 124, ncg, XL, base=1)
                        if stride != 1:
                            fw.ap[1] = [stride, ncg]
                        nc.vector.tensor_tensor(
                            out=pr[0:124, 0:ncg, 2:2 + XL],
                            in0=g3T[0:124, c0g:c0g + ncg, 2:2 + XL],
                            in1=fw, op=OP.mult)
                        ci0 = 0
                        if first:
                            nc.vector.tensor_tensor(
                                out=u3v, in0=pr[0:124, 0, 2:2 + XL],
                                in1=pr[0:124, 1, 2:2 + XL], op=OP.add)
                            first = False
                            ci0 = 2
                        for ci in range(ci0, ncg):
                            addeng(IT_GPS_MOD).tensor_tensor(
                                out=u3v, in0=u3v,
                                in1=pr[0:124, ci, 2:2 + XL], op=OP.add)

                    # P/Q -> R (PT), PQf (QT)
                    Pv = PT[0:124, 2:2 + XL]
                    Qv = QT[0:124, 2:2 + XL]
                    nc.vector.tensor_tensor(out=Pv, in0=E[0],
                                            in1=Afl[0:124, 3, 2:2 + XL],
                                            op=OP.mult)
                    for g, ch in ((1, 4), (2, 5)):
                        nc.vector.tensor_tensor(out=Cv, in0=E[g],
                                                in1=Afl[0:124, ch, 2:2 + XL],
                                                op=OP.mult)
                        addeng(IT_GPS_MOD).tensor_tensor(out=Pv, in0=Pv, in1=Cv,
                                                         op=OP.add)
                    nc.vector.tensor_scalar(out=Cv, in0=E[3],
                                            scalar1=1.0 + 1e-4, scalar2=None,
                                            op0=OP.mult)
                    nc.vector.tensor_tensor(out=Pv, in0=Pv, in1=Cv, op=OP.add)
                    nc.vector.tensor_tensor(out=Qv, in0=E[0],
                                            in1=Afl[0:124, 0, 2:2 + XL],
                                            op=OP.mult)
                    for g, ch in ((1, 1), (2, 2)):
                        nc.vector.tensor_tensor(out=Cv, in0=E[g],
                                                in1=Afl[0:124, ch, 2:2 + XL],
                                                op=OP.mult)
                        addeng(IT_GPS_MOD).tensor_tensor(out=Qv, in0=Qv, in1=Cv,
                                                         op=OP.add)
                    nc.vector.tensor_tensor(out=Qv, in0=Qv, in1=E[3], op=OP.add)
                    nc.vector.tensor_tensor(out=Qv, in0=Pv, in1=Qv,
                                            op=OP.subtract)
                    nc.vector.tensor_tensor(out=Qv, in0=Qv,
                                            in1=finT[0:124, 2:2 + XL],
                                            op=OP.mult)
                    nc.vector.reciprocal(out=Cv, in_=Pv)
                    Rv = Pv
                    nc.vector.tensor_tensor(out=Rv, in0=Cv,
                                            in1=alpT[0:124, 2:2 + XL],
                                            op=OP.mult)

                    # u2 (C2): center group full-width, others streamed halves
                    uv = u2[0:124, 2:2 + XL]
                    Cw = C2c[0:124, :, 2:2 + XL]
                    pr = nprod()
                    fw = _fwin(FS(2), 0, 124, 5, XL)
                    nc.vector.tensor_tensor(out=pr[0:124, :, 2:2 + XL],
                                            in0=Cw, in1=fw, op=OP.mult)
                    nc.vector.tensor_tensor(out=uv, in0=pr[0:124, 0, 2:2 + XL],
                                            in1=pr[0:124, 1, 2:2 + XL], op=OP.add)
                    for ci in range(2, 5):
                        addeng(IT_GPS_MOD).tensor_tensor(
                            out=uv, in0=uv, in1=pr[0:124, ci, 2:2 + XL],
                            op=OP.add)
                    ldq = [(0, 0), (0, 1), (1, 0), (1, 1), (3, 0), (3, 1),
                           (4, 0), (4, 1)]
                    nld = [2]
                    for g in (0, 1, 3, 4):
                        for h in range(2):
                            if (g, h) not in cs_t:
                                cs_load(g, h)
                                nld[0] += 1
                            # prefetch ahead
                            if nld[0] < len(ldq):
                                cs_load(*ldq[nld[0]])
                                nld[0] += 1
                            lo, nct = (1, 3) if g in (0, 4) else (0, 5)
                            oc0 = 2 if h == 0 else HXF
                            wid = HXF - 2
                            cc0 = 2 if h == 0 else 0
                            csv = cs_t[(g, h)][0:124, 0:nct, cc0:cc0 + wid]
                            pr = nprod()
                            fw = _fwin(FS(g), 0, 124, nct, wid,
                                       base=oc0 - 2 + lo)
                            nc.vector.tensor_tensor(
                                out=pr[0:124, 0:nct, oc0:oc0 + wid], in0=csv,
                                in1=fw, op=OP.mult)
                            uvh = u2[0:124, oc0:oc0 + wid]
                            for ci in range(nct):
                                addeng(IT_GPS_MOD).tensor_tensor(
                                    out=uvh, in0=uvh,
                                    in1=pr[0:124, ci, oc0:oc0 + wid], op=OP.add)

                    # combine
                    NV = num[0:124, 2:2 + XL]
                    nc.vector.tensor_tensor(out=NV, in0=E[0],
                                            in1=u1[0:124, 2:2 + XL], op=OP.mult)
                    for q, uv in ((1, u2[0:124, 2:2 + XL]), (2, u3v)):
                        nc.vector.tensor_tensor(out=Cv, in0=E[q], in1=uv,
                                                op=OP.mult)
                        addeng(IT_GPS_MOD).tensor_tensor(out=NV, in0=NV, in1=Cv,
                                                         op=OP.add)
                    nc.vector.tensor_tensor(out=Cv, in0=E[3],
                                            in1=FS(2)[0:124, 2:2 + XL], op=OP.mult)
                    nc.vector.tensor_tensor(out=NV, in0=NV, in1=Cv, op=OP.add)
                    nc.vector.tensor_tensor(out=NV, in0=NV, in1=Qv, op=OP.add)
                    nc.vector.tensor_tensor(out=NV, in0=NV, in1=Rv, op=OP.mult)
                    nc.vector.tensor_tensor(out=NV, in0=NV,
                                            in1=betT[0:124, 2:2 + XL], op=OP.add)
                    for b in range(NB):
                        dmaeng().dma_start(
                            out=nxt[2:126, b * BW + X2A:b * BW + X2B],
                            in_=num[0:124, b * BW + X2A:b * BW + X2B])
                    nc.sync.dma_start(out=nxt[126:128, 0:3 * BW],
                                      in_=nxt[2:4, BW:XF])
                    nc.scalar.dma_start(out=nxt[0:2, BW:XF],
                                        in_=nxt[124:126, 0:3 * BW])
                    cur, nxt = nxt, cur
                for b in range(NB):
                    pend = 110 if b == 3 else 126
                    nc.sync.dma_start(
                        out=outD[124 * b:124 * b + (pend - 2), :],
                        in_=cur[2:pend, b * BW + X2A:b * BW + X2B])
    _split_2d_f16(nc)
    _split_waits(nc)
    return nc


_NC_CACHE = {}


def _prep_core_inputs(inputs):
    f16 = np.float16
    bf16 = mybir.dt.np(BF16)
    W1, b1 = _pack_conv120(inputs['w_off1'], inputs['b_off1'])
    W2, b2 = _pack_conv120(inputs['w_off2'], inputs['b_off2'])
    maps = []
    for c in range(NCORE):
        bimg, half = c // 2, c % 2
        xs = 0 if half == 0 else 308
        gp = np.zeros((24, ROWS, 644), np.float32)
        gp[:, 2:482, 2:642] = inputs['guidance'][bimg]
        # row-triple im2col: g12[40*cv + 5c + j, T, x] =
        #   g(8cv+c, 124*(T//42) + 1 + 3*(T%42) + j, x)
        gsl = gp[0:16, :, xs:xs + BW]
        Tn = TRI * NB
        Ti = np.arange(Tn)
        g12f = np.zeros((80, Tn, BW), np.float32)
        for cv in range(2):
            for c in range(8):
                for j in range(5):
                    rows = 124 * (Ti // TRI) + 1 + 3 * (Ti % TRI) + j
                    g12f[40 * cv + 5 * c + j] = gsl[8 * cv + c, rows, :]
        g12 = np.ascontiguousarray(
            np.stack([g12f[:, :, d:d + XWID] for d in range(3)])).astype(bf16)
        g3p = np.pad(gp[16:24], ((0, 0), (1, 1), (1, 1)))
        g3s = np.zeros((ROWS, 8, BW), f16)
        NEWSH = [(1, -1), (1, 0), (1, 1), (0, -1), (0, 1),
                 (-1, -1), (-1, 0), (-1, 1)]
        for i, (sdy, sdx) in enumerate(NEWSH):
            ch = SH.index((sdy, sdx))
            g3s[:, i, :] = g3p[ch, 1 + sdy:1 + sdy + ROWS,
                               1 + xs + sdx:1 + xs + sdx + BW]
        dp = np.zeros((24, ROWS, 644), np.float32)
        dp[:, 2:482, 2:642] = inputs['dynamic'][bimg]
        dyn = np.exp(np.ascontiguousarray(
            dp[:, :, xs:xs + BW].reshape(PROP, 4, ROWS, BW)
            .transpose(0, 2, 1, 3))).astype(f16)
        fp = np.zeros((ROWS, 3, 644), np.float32)
        fp[2:482, 0, 2:642] = inputs['feat_init'][bimg, 0]
        fp[2:482, 1, 2:642] = inputs['confidence'][bimg, 0]
        fp[2:482, 2, 2:642] = inputs['feat_fix'][bimg, 0]
        fcf = np.ascontiguousarray(fp[:, :, xs:xs + BW])
        fnh = np.ascontiguousarray(fp[:, 0, xs:xs + BW]).astype(f16)
        maps.append({
            "g12": g12, "g3s": g3s, "dyn": dyn, "fcf": fcf, "fnh": fnh,
            "w1": W1.astype(bf16), "w2": W2.astype(bf16),
            "b1": b1, "b2": b2,
        })
    return maps


def run_cores(inputs, trace=False):
    if 'nc' not in _NC_CACHE:
        _NC_CACHE['nc'] = build_nc()
    nc = _NC_CACHE['nc']
    maps = _prep_core_inputs(inputs)
    res = bass_utils.run_bass_kernel_spmd(nc, maps, core_ids=list(range(NCORE)),
                                          trace=trace)
    out = np.zeros((B, 1, H, W), np.float32)
    for c in range(NCORE):
        bimg, half = c // 2, c % 2
        o = res.results[c]["out"].astype(np.float32)
        if half == 0:
            out[bimg, 0, :, 0:320] = o[:, 0:320]
        else:
            out[bimg, 0, :, 320:640] = o[:, 12:332]
    return out, res


def kernel(**inputs):
    out, _ = run_cores(inputs, trace=False)
    return out


if __name__ == "__main__":
    import pickle
    with open('/tmp/inputs.pkl', 'rb') as f:
        inputs = pickle.load(f)
    ref = np.load('/tmp/ref_out.npy')
    got, res = run_cores(inputs, trace=False)
    rel = np.linalg.norm(got - ref) / np.linalg.norm(ref)
    print("Relative error:", rel, " absmax:", np.abs(got - ref).max())
